# revision 23
# baseline (speedup 1.0000x reference)
"""Pairwise squared-distance kernel for Trainium2 (8 NeuronCores).

out[i, j] = mean_d (x_i[d] - y_j[d])^2
          = (||x_i||^2 + ||y_j||^2 - 2 x_i . y_j) / D

Sharding: rows of z_queries split across 8 cores (1024 rows each);
class_prototypes replicated. Each core computes its [1024, 4096] slab.

Device kernel (per core):
  - inputs pre-transposed on host to [D, rows] so the contraction dim is
    the SBUF partition dim (clean contiguous DMAs, no on-chip transpose).
  - prototypes pre-scaled by -2/D (= -2^-8, exact) so PSUM accumulates
    -2/D * x.y directly.
  - GEMM: for each (m-tile 128 queries, n-half 2048 protos): 4 k-tiles x
    4 n-subtiles of matmul into a [128, 2048] PSUM tile (4 banks).
  - epilogue: one DVE scalar_tensor_tensor: (psum + xsq/D[i]) + ysq/D[j].
  - 1 MiB output DMAs.
"""

import sys

if "/opt/trn_rl_repo" not in sys.path:
    sys.path.insert(0, "/opt/trn_rl_repo")

import numpy as np

N_CORES = 8
N_Q = 8192
N_P = 4096
D = 512
ROWS = N_Q // N_CORES  # 1024 query rows per core

P = 128
M_TILES = ROWS // P  # 8
K_TILES = D // P  # 4
N_BLOCK = 1024  # psum group free dim (2 banks of fp32)
N_BLOCKS = N_P // N_BLOCK  # 4
NB = 512  # matmul free dim (1 psum bank)
NSUB = N_BLOCK // NB  # 2
WAVE = 4  # m-tiles per wave (4 x 2 banks = 8 PSUM banks)
N_WAVES = M_TILES // WAVE  # 2

# "bf16" halves input DMA traffic; "f32r" keeps fp32 inputs at TF32 matmul rate.
COMPUTE_DT = "bf16"

_CACHE = {}


def _build_nc(compute_dt: str):
    import concourse.mybir as mybir
    import concourse.tile as tile
    from concourse import bacc

    if compute_dt == "bf16":
        in_dt = mybir.dt.bfloat16
        mm_cast = lambda ap: ap
    elif compute_dt == "f32r":
        in_dt = mybir.dt.float32
        mm_cast = lambda ap: ap.bitcast(mybir.dt.float32r)
    else:
        raise ValueError(compute_dt)

    f32 = mybir.dt.float32
    add = mybir.AluOpType.add

    nc = bacc.Bacc("TRN2", target_bir_lowering=False, debug=False, num_devices=N_CORES)

    # qp packs [qt | pt] along the free dim so one DMA chunk [qt_k | pt_nb0_k]
    # unlocks the first k-sweep with a single completion.
    qp = nc.dram_tensor("qp", (D, ROWS + N_P), in_dt, kind="ExternalInput")
    ab = nc.dram_tensor("ab", (P, M_TILES), f32, kind="ExternalInput")
    bb = nc.dram_tensor("bb", (1, N_P), f32, kind="ExternalInput")
    out = nc.dram_tensor("out", (ROWS, N_P), f32, kind="ExternalOutput")
    N_FRONT = ROWS + N_BLOCK  # 2048: qt_k | pt_nb0_k
    N_REST = N_P - N_BLOCK  # 3072: pt_nb1..3_k

    with tile.TileContext(nc) as tc:
        with (
            tc.tile_pool(name="inputs", bufs=1) as in_pool,
            tc.tile_pool(name="outs", bufs=8) as out_pool,
            tc.tile_pool(name="psum", bufs=WAVE, space="PSUM") as psum_pool,
        ):
            # Input loads split across both HWDGE rings (sync + scalar),
            # interleaved in compute-consumption order so the PE goes dense
            # as soon as the first (qt_k0, pt_b0_k0) chunks land.
            # ring A (scalar): qt per-k chunks, ab, b row
            # ring S (sync): pt per-(block, k) chunks
            # Inputs ride the sync HWDGE ring (q1) AND the gpsimd SWDGE queues
            # in consumption order — q1 alone tops out ~220 GB/s on reads, and
            # the scalar ring (q10) is starved by q1, but SWDGE rows arbitrate
            # fairly against q1 at the SDMA engines. Outputs go to the scalar
            # ring, which naturally yields to input traffic.
            qt_tiles = [None] * K_TILES
            pt_tiles = [[None] * K_TILES for _ in range(N_BLOCKS)]
            rest_tiles = [None] * K_TILES

            def load_front(k, eng):
                fr_t = in_pool.tile([P, N_FRONT], in_dt, name=f"front_{k}")
                eng.dma_start(out=fr_t, in_=qp[k * P : (k + 1) * P, 0:N_FRONT])
                qt_tiles[k] = fr_t[:, 0:ROWS]
                pt_tiles[0][k] = fr_t[:, ROWS:N_FRONT]

            def load_rest(k, eng):
                re_t = in_pool.tile([P, N_REST], in_dt, name=f"rest_{k}")
                eng.dma_start(
                    out=re_t, in_=qp[k * P : (k + 1) * P, N_FRONT : ROWS + N_P]
                )
                rest_tiles[k] = re_t
                for nb in range(1, N_BLOCKS):
                    pt_tiles[nb][k] = re_t[:, (nb - 1) * N_BLOCK : nb * N_BLOCK]

            brow_t = in_pool.tile([1, N_P], f32, name="brow_t")
            nc.sync.dma_start(out=brow_t, in_=bb[0:1, :])
            for k in range(K_TILES):
                load_front(k, nc.sync if k % 2 == 0 else nc.gpsimd)
            ab_t = in_pool.tile([P, M_TILES], f32, name="ab_t")
            nc.sync.dma_start(out=ab_t, in_=ab[:, :])
            for k in range(K_TILES):
                load_rest(k, nc.sync if k % 2 == 0 else nc.gpsimd)
            # b-row partition broadcast (saves a 2 MiB HBM load); emitted after
            # the gpsimd-issued input DMAs so it doesn't block their issue.
            bb_t = in_pool.tile([P, N_P], f32, name="bb_t")
            nc.gpsimd.partition_broadcast(bb_t, brow_t)

            n_out = 0

            def epilogue(psum_t, m, nb):
                nonlocal n_out
                out_t = out_pool.tile([P, N_BLOCK], f32, name="out_t")
                # out = (psum + xsq/D[i]) + ysq/D[j]
                nc.vector.scalar_tensor_tensor(
                    out=out_t,
                    in0=psum_t,
                    scalar=ab_t[:, m : m + 1],
                    in1=bb_t[:, nb * N_BLOCK : (nb + 1) * N_BLOCK],
                    op0=add,
                    op1=add,
                )
                out_eng = nc.scalar if n_out % 2 == 0 else nc.sync
                n_out += 1
                out_eng.dma_start(
                    out=out[m * P : (m + 1) * P, nb * N_BLOCK : (nb + 1) * N_BLOCK],
                    in_=out_t,
                )

            def mm(psum_t, m, nb, k, start, stop):
                lhsT = mm_cast(qt_tiles[k][:, m * P : (m + 1) * P])
                for ns in range(NSUB):
                    nc.tensor.matmul(
                        psum_t[:, ns * NB : (ns + 1) * NB],
                        lhsT,
                        mm_cast(pt_tiles[nb][k][:, ns * NB : (ns + 1) * NB]),
                        start=start,
                        stop=stop,
                    )

            # nb0: k-outer / m-inner waves — each newly-landed (qt_k, pt_k)
            # chunk pair unlocks a full 8-matmul sweep, so the PE goes dense
            # while inputs are still streaming in.
            for w in range(N_WAVES):
                psums = [
                    psum_pool.tile([P, N_BLOCK], f32, name="ps", tag="ps")
                    for _ in range(WAVE)
                ]
                for k in range(K_TILES):
                    for mi in range(WAVE):
                        mm(psums[mi], w * WAVE + mi, 0, k, k == 0, k == K_TILES - 1)
                for mi in range(WAVE):
                    epilogue(psums[mi], w * WAVE + mi, 0)

            # nb1..3: everything is resident by now — m-outer / k-inner, so
            # each m-group's epilogue pipelines under the next group's matmuls
            # (the kernel tail is one epilogue, not a whole wave of them).
            for nb in range(1, N_BLOCKS):
                for m in range(M_TILES):
                    psum_t = psum_pool.tile([P, N_BLOCK], f32, name="ps", tag="ps")
                    for k in range(K_TILES):
                        mm(psum_t, m, nb, k, k == 0, k == K_TILES - 1)
                    epilogue(psum_t, m, nb)

    nc.compile()
    return nc


def _get_nc(compute_dt: str):
    if compute_dt not in _CACHE:
        _CACHE[compute_dt] = _build_nc(compute_dt)
    return _CACHE[compute_dt]


def _prep_inputs(z_queries: np.ndarray, class_prototypes: np.ndarray, compute_dt: str):
    import ml_dtypes

    np_in = ml_dtypes.bfloat16 if compute_dt == "bf16" else np.float32

    z = np.ascontiguousarray(z_queries, dtype=np.float32)
    p = np.ascontiguousarray(class_prototypes, dtype=np.float32)

    a = (z.astype(np.float64) ** 2).sum(axis=1) / D  # (N_Q,) ||x||^2 / D
    b = (p.astype(np.float64) ** 2).sum(axis=1) / D  # (N_P,) ||y||^2 / D

    pt = (p.T * np.float32(-2.0 / D)).astype(np_in)  # [D, N_P]
    bb = np.ascontiguousarray(b.astype(np.float32).reshape(1, N_P))  # [1, N_P]

    in_maps = []
    for c in range(N_CORES):
        sl = slice(c * ROWS, (c + 1) * ROWS)
        qt_c = z[sl].T.astype(np_in)  # [D, ROWS]
        qp_c = np.ascontiguousarray(np.concatenate([qt_c, pt], axis=1))  # [D, ROWS+N_P]
        ab_c = np.ascontiguousarray(
            a[sl].astype(np.float32).reshape(M_TILES, P).T
        )  # [P, M_TILES]
        in_maps.append({"qp": qp_c, "ab": ab_c, "bb": bb})
    return in_maps


def run(z_queries, class_prototypes, compute_dt=COMPUTE_DT, **spmd_kwargs):
    from concourse.bass_utils import run_bass_kernel_spmd

    nc = _get_nc(compute_dt)
    in_maps = _prep_inputs(z_queries, class_prototypes, compute_dt)
    res = run_bass_kernel_spmd(nc, in_maps, core_ids=list(range(N_CORES)), **spmd_kwargs)
    full = np.concatenate([r["out"] for r in res.results], axis=0)
    return full, res


def kernel(z_queries: np.ndarray, class_prototypes: np.ndarray) -> np.ndarray:
    full, _ = run(z_queries, class_prototypes)
    return full


# revision 24
# speedup vs baseline: 1.0651x; 1.0651x over previous
"""Pairwise squared-distance kernel for Trainium2 (8 NeuronCores).

out[i, j] = mean_d (x_i[d] - y_j[d])^2
          = (||x_i||^2 + ||y_j||^2 - 2 x_i . y_j) / D

Sharding: rows of z_queries split across 8 cores (1024 rows each);
class_prototypes replicated. Each core computes its [1024, 4096] slab.

Device kernel (per core):
  - inputs pre-transposed on host to [D, rows] so the contraction dim is
    the SBUF partition dim (clean contiguous DMAs, no on-chip transpose).
  - prototypes pre-scaled by -2/D (= -2^-8, exact) so PSUM accumulates
    -2/D * x.y directly.
  - GEMM: for each (m-tile 128 queries, n-half 2048 protos): 4 k-tiles x
    4 n-subtiles of matmul into a [128, 2048] PSUM tile (4 banks).
  - epilogue: one DVE scalar_tensor_tensor: (psum + xsq/D[i]) + ysq/D[j].
  - 1 MiB output DMAs.
"""

import sys

if "/opt/trn_rl_repo" not in sys.path:
    sys.path.insert(0, "/opt/trn_rl_repo")

import numpy as np

N_CORES = 8
N_Q = 8192
N_P = 4096
D = 512
ROWS = N_Q // N_CORES  # 1024 query rows per core

P = 128
M_TILES = ROWS // P  # 8
K_TILES = D // P  # 4
N_BLOCK = 1024  # psum group free dim (2 banks of fp32)
N_BLOCKS = N_P // N_BLOCK  # 4
NB = 512  # matmul free dim (1 psum bank)
NSUB = N_BLOCK // NB  # 2
WAVE = 4  # m-tiles per wave (4 x 2 banks = 8 PSUM banks)
N_WAVES = M_TILES // WAVE  # 2

# "bf16" halves input DMA traffic; "f32r" keeps fp32 inputs at TF32 matmul rate.
COMPUTE_DT = "bf16"

_CACHE = {}


def _build_nc(compute_dt: str):
    import concourse.mybir as mybir
    import concourse.tile as tile
    from concourse import bacc

    if compute_dt == "bf16":
        in_dt = mybir.dt.bfloat16
        mm_cast = lambda ap: ap
    elif compute_dt == "f32r":
        in_dt = mybir.dt.float32
        mm_cast = lambda ap: ap.bitcast(mybir.dt.float32r)
    else:
        raise ValueError(compute_dt)

    f32 = mybir.dt.float32
    add = mybir.AluOpType.add

    nc = bacc.Bacc("TRN2", target_bir_lowering=False, debug=False, num_devices=N_CORES)

    # qp packs [qt | pt] along the free dim so one DMA chunk [qt_k | pt_nb0_k]
    # unlocks the first k-sweep with a single completion.
    qp = nc.dram_tensor("qp", (D, ROWS + N_P), in_dt, kind="ExternalInput")
    ab = nc.dram_tensor("ab", (P, M_TILES), f32, kind="ExternalInput")
    bb = nc.dram_tensor("bb", (1, N_P), f32, kind="ExternalInput")
    out = nc.dram_tensor("out", (ROWS, N_P), f32, kind="ExternalOutput")
    N_FRONT = ROWS + N_BLOCK  # 2048: qt_k | pt_nb0_k
    N_REST = N_P - N_BLOCK  # 3072: pt_nb1..3_k

    with tile.TileContext(nc) as tc:
        with (
            tc.tile_pool(name="inputs", bufs=1) as in_pool,
            tc.tile_pool(name="outs", bufs=8) as out_pool,
            tc.tile_pool(name="psum", bufs=WAVE, space="PSUM") as psum_pool,
        ):
            # Input loads split across both HWDGE rings (sync + scalar),
            # interleaved in compute-consumption order so the PE goes dense
            # as soon as the first (qt_k0, pt_b0_k0) chunks land.
            # ring A (scalar): qt per-k chunks, ab, b row
            # ring S (sync): pt per-(block, k) chunks
            # Inputs ride the sync HWDGE ring (q1) AND the gpsimd SWDGE queues
            # in consumption order — q1 alone tops out ~220 GB/s on reads, and
            # the scalar ring (q10) is starved by q1, but SWDGE rows arbitrate
            # fairly against q1 at the SDMA engines. Outputs go to the scalar
            # ring, which naturally yields to input traffic.
            qt_tiles = [None] * K_TILES
            pt_tiles = [[None] * K_TILES for _ in range(N_BLOCKS)]
            rest_tiles = [None] * K_TILES

            def load_front(k, eng):
                fr_t = in_pool.tile([P, N_FRONT], in_dt, name=f"front_{k}")
                eng.dma_start(out=fr_t, in_=qp[k * P : (k + 1) * P, 0:N_FRONT])
                qt_tiles[k] = fr_t[:, 0:ROWS]
                pt_tiles[0][k] = fr_t[:, ROWS:N_FRONT]

            def load_rest(k, eng):
                re_t = in_pool.tile([P, N_REST], in_dt, name=f"rest_{k}")
                eng.dma_start(
                    out=re_t, in_=qp[k * P : (k + 1) * P, N_FRONT : ROWS + N_P]
                )
                rest_tiles[k] = re_t
                for nb in range(1, N_BLOCKS):
                    pt_tiles[nb][k] = re_t[:, (nb - 1) * N_BLOCK : nb * N_BLOCK]

            load_front(0, nc.sync)
            # b row early (tiny); its on-device partition broadcast (saves a
            # 2 MiB HBM load) runs on GpSimd during the input stream.
            brow_t = in_pool.tile([1, N_P], f32, name="brow_t")
            nc.sync.dma_start(out=brow_t, in_=bb[0:1, :])
            bb_t = in_pool.tile([P, N_P], f32, name="bb_t")
            nc.gpsimd.partition_broadcast(bb_t, brow_t)
            for k in range(1, K_TILES):
                load_front(k, nc.sync)
            ab_t = in_pool.tile([P, M_TILES], f32, name="ab_t")
            nc.sync.dma_start(out=ab_t, in_=ab[:, :])
            for k in range(K_TILES):
                load_rest(k, nc.sync)

            n_out = 0

            def epilogue(psum_t, m, nb):
                nonlocal n_out
                out_t = out_pool.tile([P, N_BLOCK], f32, name="out_t")
                # out = (psum + xsq/D[i]) + ysq/D[j]
                nc.vector.scalar_tensor_tensor(
                    out=out_t,
                    in0=psum_t,
                    scalar=ab_t[:, m : m + 1],
                    in1=bb_t[:, nb * N_BLOCK : (nb + 1) * N_BLOCK],
                    op0=add,
                    op1=add,
                )
                out_eng = nc.scalar if n_out % 2 == 0 else nc.sync
                n_out += 1
                out_eng.dma_start(
                    out=out[m * P : (m + 1) * P, nb * N_BLOCK : (nb + 1) * N_BLOCK],
                    in_=out_t,
                )

            def mm(psum_t, m, nb, k, start, stop):
                lhsT = mm_cast(qt_tiles[k][:, m * P : (m + 1) * P])
                for ns in range(NSUB):
                    nc.tensor.matmul(
                        psum_t[:, ns * NB : (ns + 1) * NB],
                        lhsT,
                        mm_cast(pt_tiles[nb][k][:, ns * NB : (ns + 1) * NB]),
                        start=start,
                        stop=stop,
                    )

            # nb0: k-outer / m-inner waves — each newly-landed (qt_k, pt_k)
            # chunk pair unlocks a full 8-matmul sweep, so the PE goes dense
            # while inputs are still streaming in.
            for w in range(N_WAVES):
                psums = [
                    psum_pool.tile([P, N_BLOCK], f32, name="ps", tag="ps")
                    for _ in range(WAVE)
                ]
                for k in range(K_TILES):
                    for mi in range(WAVE):
                        mm(psums[mi], w * WAVE + mi, 0, k, k == 0, k == K_TILES - 1)
                for mi in range(WAVE):
                    epilogue(psums[mi], w * WAVE + mi, 0)

            # nb1..3: everything is resident by now — m-outer / k-inner, so
            # each m-group's epilogue pipelines under the next group's matmuls
            # (the kernel tail is one epilogue, not a whole wave of them).
            for nb in range(1, N_BLOCKS):
                for m in range(M_TILES):
                    psum_t = psum_pool.tile([P, N_BLOCK], f32, name="ps", tag="ps")
                    for k in range(K_TILES):
                        mm(psum_t, m, nb, k, k == 0, k == K_TILES - 1)
                    epilogue(psum_t, m, nb)

    nc.compile()
    return nc


def _get_nc(compute_dt: str):
    if compute_dt not in _CACHE:
        _CACHE[compute_dt] = _build_nc(compute_dt)
    return _CACHE[compute_dt]


def _prep_inputs(z_queries: np.ndarray, class_prototypes: np.ndarray, compute_dt: str):
    import ml_dtypes

    np_in = ml_dtypes.bfloat16 if compute_dt == "bf16" else np.float32

    z = np.ascontiguousarray(z_queries, dtype=np.float32)
    p = np.ascontiguousarray(class_prototypes, dtype=np.float32)

    a = (z.astype(np.float64) ** 2).sum(axis=1) / D  # (N_Q,) ||x||^2 / D
    b = (p.astype(np.float64) ** 2).sum(axis=1) / D  # (N_P,) ||y||^2 / D

    pt = (p.T * np.float32(-2.0 / D)).astype(np_in)  # [D, N_P]
    bb = np.ascontiguousarray(b.astype(np.float32).reshape(1, N_P))  # [1, N_P]

    in_maps = []
    for c in range(N_CORES):
        sl = slice(c * ROWS, (c + 1) * ROWS)
        qt_c = z[sl].T.astype(np_in)  # [D, ROWS]
        qp_c = np.ascontiguousarray(np.concatenate([qt_c, pt], axis=1))  # [D, ROWS+N_P]
        ab_c = np.ascontiguousarray(
            a[sl].astype(np.float32).reshape(M_TILES, P).T
        )  # [P, M_TILES]
        in_maps.append({"qp": qp_c, "ab": ab_c, "bb": bb})
    return in_maps


def run(z_queries, class_prototypes, compute_dt=COMPUTE_DT, **spmd_kwargs):
    from concourse.bass_utils import run_bass_kernel_spmd

    nc = _get_nc(compute_dt)
    in_maps = _prep_inputs(z_queries, class_prototypes, compute_dt)
    res = run_bass_kernel_spmd(nc, in_maps, core_ids=list(range(N_CORES)), **spmd_kwargs)
    full = np.concatenate([r["out"] for r in res.results], axis=0)
    return full, res


def kernel(z_queries: np.ndarray, class_prototypes: np.ndarray) -> np.ndarray:
    full, _ = run(z_queries, class_prototypes)
    return full


# revision 25
# speedup vs baseline: 1.0820x; 1.0159x over previous
"""Pairwise squared-distance kernel for Trainium2 (8 NeuronCores).

out[i, j] = mean_d (x_i[d] - y_j[d])^2
          = (||x_i||^2 + ||y_j||^2 - 2 x_i . y_j) / D

Sharding: rows of z_queries split across 8 cores (1024 rows each);
class_prototypes replicated. Each core computes its [1024, 4096] slab.

Device kernel (per core):
  - inputs pre-transposed on host to [D, rows] so the contraction dim is
    the SBUF partition dim (clean contiguous DMAs, no on-chip transpose).
  - prototypes pre-scaled by -2/D (= -2^-8, exact) so PSUM accumulates
    -2/D * x.y directly.
  - GEMM: for each (m-tile 128 queries, n-half 2048 protos): 4 k-tiles x
    4 n-subtiles of matmul into a [128, 2048] PSUM tile (4 banks).
  - epilogue: one DVE scalar_tensor_tensor: (psum + xsq/D[i]) + ysq/D[j].
  - 1 MiB output DMAs.
"""

import sys

if "/opt/trn_rl_repo" not in sys.path:
    sys.path.insert(0, "/opt/trn_rl_repo")

import numpy as np

N_CORES = 8
N_Q = 8192
N_P = 4096
D = 512
ROWS = N_Q // N_CORES  # 1024 query rows per core

P = 128
M_TILES = ROWS // P  # 8
K_TILES = D // P  # 4
N_BLOCK = 1024  # psum group free dim (2 banks of fp32)
N_BLOCKS = N_P // N_BLOCK  # 4
NB = 512  # matmul free dim (1 psum bank)
NSUB = N_BLOCK // NB  # 2
WAVE = 4  # m-tiles per wave (4 x 2 banks = 8 PSUM banks)
N_WAVES = M_TILES // WAVE  # 2

# "bf16" halves input DMA traffic; "f32r" keeps fp32 inputs at TF32 matmul rate.
COMPUTE_DT = "bf16"

_CACHE = {}


def _build_nc(compute_dt: str):
    import concourse.mybir as mybir
    import concourse.tile as tile
    from concourse import bacc

    if compute_dt == "bf16":
        in_dt = mybir.dt.bfloat16
        mm_cast = lambda ap: ap
    elif compute_dt == "f32r":
        in_dt = mybir.dt.float32
        mm_cast = lambda ap: ap.bitcast(mybir.dt.float32r)
    else:
        raise ValueError(compute_dt)

    f32 = mybir.dt.float32
    add = mybir.AluOpType.add

    nc = bacc.Bacc("TRN2", target_bir_lowering=False, debug=False, num_devices=N_CORES)

    # qp packs [qt | pt] along the free dim so one DMA chunk [qt_k | pt_nb0_k]
    # unlocks the first k-sweep with a single completion.
    qp = nc.dram_tensor("qp", (D, ROWS + N_P), in_dt, kind="ExternalInput")
    ab = nc.dram_tensor("ab", (P, M_TILES), f32, kind="ExternalInput")
    bb = nc.dram_tensor("bb", (1, N_P), f32, kind="ExternalInput")
    out = nc.dram_tensor("out", (ROWS, N_P), f32, kind="ExternalOutput")
    N_FRONT = ROWS + NB  # 1536: qt_k | pt_block0_k
    N_REST = N_P - 2 * NB  # 3072: pt blocks 2..7
    NBLK = N_P // NB  # 8 column blocks of 512

    with tile.TileContext(nc) as tc:
        with (
            tc.tile_pool(name="inputs", bufs=1) as in_pool,
            tc.tile_pool(name="outs", bufs=8) as out_pool,
            tc.tile_pool(name="psum", bufs=8, space="PSUM") as psum_pool,
        ):
            # All inputs ride the sync ring (q1) in exact consumption order —
            # the two HWDGE rings don't round-robin fairly (q1 starves q10),
            # so FIFO position on q1 IS the data priority. Outputs go to the
            # scalar ring (q10), which naturally yields to input traffic.
            qt_tiles = [None] * K_TILES
            ptb = [[None] * K_TILES for _ in range(NBLK)]

            def load_front(k):
                fr_t = in_pool.tile([P, N_FRONT], in_dt, name=f"front_{k}")
                nc.sync.dma_start(out=fr_t, in_=qp[k * P : (k + 1) * P, 0:N_FRONT])
                qt_tiles[k] = fr_t[:, 0:ROWS]
                ptb[0][k] = fr_t[:, ROWS:N_FRONT]

            def load_b1(k):
                b1_t = in_pool.tile([P, NB], in_dt, name=f"b1_{k}")
                nc.sync.dma_start(
                    out=b1_t, in_=qp[k * P : (k + 1) * P, N_FRONT : N_FRONT + NB]
                )
                ptb[1][k] = b1_t

            def load_rest(k):
                re_t = in_pool.tile([P, N_REST], in_dt, name=f"rest_{k}")
                nc.sync.dma_start(
                    out=re_t, in_=qp[k * P : (k + 1) * P, N_FRONT + NB : ROWS + N_P]
                )
                for b in range(2, NBLK):
                    ptb[b][k] = re_t[:, (b - 2) * NB : (b - 1) * NB]

            load_front(0)
            # b row early (tiny); its on-device partition broadcast (saves a
            # 2 MiB HBM load) runs on GpSimd during the input stream.
            brow_t = in_pool.tile([1, N_P], f32, name="brow_t")
            nc.sync.dma_start(out=brow_t, in_=bb[0:1, :])
            bb_t = in_pool.tile([P, N_P], f32, name="bb_t")
            nc.gpsimd.partition_broadcast(bb_t, brow_t)
            for k in range(1, K_TILES):
                load_front(k)
            for k in range(K_TILES):
                load_b1(k)
            ab_t = in_pool.tile([P, M_TILES], f32, name="ab_t")
            nc.sync.dma_start(out=ab_t, in_=ab[:, :])
            for k in range(K_TILES):
                load_rest(k)

            n_out = 0

            def epilogue(psum_t, m, b):
                nonlocal n_out
                out_t = out_pool.tile([P, NB], f32, name="out_t")
                # out = (psum + xsq/D[i]) + ysq/D[j]
                nc.vector.scalar_tensor_tensor(
                    out=out_t,
                    in0=psum_t,
                    scalar=ab_t[:, m : m + 1],
                    in1=bb_t[:, b * NB : (b + 1) * NB],
                    op0=add,
                    op1=add,
                )
                out_eng = nc.scalar if n_out % 2 == 0 else nc.sync
                n_out += 1
                out_eng.dma_start(
                    out=out[m * P : (m + 1) * P, b * NB : (b + 1) * NB],
                    in_=out_t,
                )

            def mm(psum_t, m, b, k):
                nc.tensor.matmul(
                    psum_t,
                    mm_cast(qt_tiles[k][:, m * P : (m + 1) * P]),
                    mm_cast(ptb[b][k]),
                    start=(k == 0),
                    stop=(k == K_TILES - 1),
                )

            # Blocks 0-1: k-outer / m-inner over all 8 m-tiles (8 one-bank
            # PSUM groups) — each newly-landed chunk unlocks a full 8-matmul
            # sweep, so the PE goes dense while inputs are still streaming.
            for b in range(2):
                psums = [
                    psum_pool.tile([P, NB], f32, name="ps", tag="ps")
                    for _ in range(M_TILES)
                ]
                for k in range(K_TILES):
                    for m in range(M_TILES):
                        mm(psums[m], m, b, k)
                for m in range(M_TILES):
                    epilogue(psums[m], m, b)

            # Blocks 2-7: everything is resident — m-outer / k-inner, so each
            # group's epilogue pipelines under the next group's matmuls and
            # the kernel tail is a single small epilogue + 256 KiB store.
            for b in range(2, NBLK):
                for m in range(M_TILES):
                    psum_t = psum_pool.tile([P, NB], f32, name="ps", tag="ps")
                    for k in range(K_TILES):
                        mm(psum_t, m, b, k)
                    epilogue(psum_t, m, b)

    nc.compile()
    return nc


def _get_nc(compute_dt: str):
    if compute_dt not in _CACHE:
        _CACHE[compute_dt] = _build_nc(compute_dt)
    return _CACHE[compute_dt]


def _prep_inputs(z_queries: np.ndarray, class_prototypes: np.ndarray, compute_dt: str):
    import ml_dtypes

    np_in = ml_dtypes.bfloat16 if compute_dt == "bf16" else np.float32

    z = np.ascontiguousarray(z_queries, dtype=np.float32)
    p = np.ascontiguousarray(class_prototypes, dtype=np.float32)

    a = (z.astype(np.float64) ** 2).sum(axis=1) / D  # (N_Q,) ||x||^2 / D
    b = (p.astype(np.float64) ** 2).sum(axis=1) / D  # (N_P,) ||y||^2 / D

    pt = (p.T * np.float32(-2.0 / D)).astype(np_in)  # [D, N_P]
    bb = np.ascontiguousarray(b.astype(np.float32).reshape(1, N_P))  # [1, N_P]

    in_maps = []
    for c in range(N_CORES):
        sl = slice(c * ROWS, (c + 1) * ROWS)
        qt_c = z[sl].T.astype(np_in)  # [D, ROWS]
        qp_c = np.ascontiguousarray(np.concatenate([qt_c, pt], axis=1))  # [D, ROWS+N_P]
        ab_c = np.ascontiguousarray(
            a[sl].astype(np.float32).reshape(M_TILES, P).T
        )  # [P, M_TILES]
        in_maps.append({"qp": qp_c, "ab": ab_c, "bb": bb})
    return in_maps


def run(z_queries, class_prototypes, compute_dt=COMPUTE_DT, **spmd_kwargs):
    from concourse.bass_utils import run_bass_kernel_spmd

    nc = _get_nc(compute_dt)
    in_maps = _prep_inputs(z_queries, class_prototypes, compute_dt)
    res = run_bass_kernel_spmd(nc, in_maps, core_ids=list(range(N_CORES)), **spmd_kwargs)
    full = np.concatenate([r["out"] for r in res.results], axis=0)
    return full, res


def kernel(z_queries: np.ndarray, class_prototypes: np.ndarray) -> np.ndarray:
    full, _ = run(z_queries, class_prototypes)
    return full


# revision 26
# speedup vs baseline: 1.1386x; 1.0522x over previous
"""Pairwise squared-distance kernel for Trainium2 (8 NeuronCores).

out[i, j] = mean_d (x_i[d] - y_j[d])^2
          = (||x_i||^2 + ||y_j||^2 - 2 x_i . y_j) / D

Sharding: rows of z_queries split across 8 cores (1024 rows each);
class_prototypes replicated. Each core computes its [1024, 4096] slab.

Device kernel (per core):
  - inputs pre-transposed on host to [D, rows] so the contraction dim is
    the SBUF partition dim (clean contiguous DMAs, no on-chip transpose).
  - prototypes pre-scaled by -2/D (= -2^-8, exact) so PSUM accumulates
    -2/D * x.y directly.
  - GEMM: for each (m-tile 128 queries, n-half 2048 protos): 4 k-tiles x
    4 n-subtiles of matmul into a [128, 2048] PSUM tile (4 banks).
  - epilogue: one DVE scalar_tensor_tensor: (psum + xsq/D[i]) + ysq/D[j].
  - 1 MiB output DMAs.
"""

import sys

if "/opt/trn_rl_repo" not in sys.path:
    sys.path.insert(0, "/opt/trn_rl_repo")

import numpy as np

N_CORES = 8
N_Q = 8192
N_P = 4096
D = 512
ROWS = N_Q // N_CORES  # 1024 query rows per core

P = 128
M_TILES = ROWS // P  # 8
K_TILES = D // P  # 4
N_BLOCK = 1024  # psum group free dim (2 banks of fp32)
N_BLOCKS = N_P // N_BLOCK  # 4
NB = 512  # matmul free dim (1 psum bank)
NSUB = N_BLOCK // NB  # 2
WAVE = 4  # m-tiles per wave (4 x 2 banks = 8 PSUM banks)
N_WAVES = M_TILES // WAVE  # 2

# "bf16" halves input DMA traffic; "f32r" keeps fp32 inputs at TF32 matmul rate.
COMPUTE_DT = "bf16"

_CACHE = {}


def _build_nc(compute_dt: str):
    import concourse.mybir as mybir
    import concourse.tile as tile
    from concourse import bacc

    if compute_dt == "bf16":
        in_dt = mybir.dt.bfloat16
        mm_cast = lambda ap: ap
    elif compute_dt == "f32r":
        in_dt = mybir.dt.float32
        mm_cast = lambda ap: ap.bitcast(mybir.dt.float32r)
    else:
        raise ValueError(compute_dt)

    f32 = mybir.dt.float32
    add = mybir.AluOpType.add

    nc = bacc.Bacc("TRN2", target_bir_lowering=False, debug=False, num_devices=N_CORES)

    # qp packs [qt | pt] along the free dim so one DMA chunk [qt_k | pt_nb0_k]
    # unlocks the first k-sweep with a single completion.
    qp = nc.dram_tensor("qp", (D, ROWS + N_P), in_dt, kind="ExternalInput")
    ab = nc.dram_tensor("ab", (P, M_TILES), f32, kind="ExternalInput")
    bb = nc.dram_tensor("bb", (1, N_P), f32, kind="ExternalInput")
    out = nc.dram_tensor("out", (ROWS, N_P), f32, kind="ExternalOutput")
    N_FRONT = ROWS + NB  # 1536: qt_k | pt_block0_k
    N_REST = N_P - 2 * NB  # 3072: pt blocks 2..7
    NBLK = N_P // NB  # 8 column blocks of 512

    with tile.TileContext(nc) as tc:
        with (
            tc.tile_pool(name="inputs", bufs=1) as in_pool,
            tc.tile_pool(name="outs", bufs=8) as out_pool,
            tc.tile_pool(name="psum", bufs=8, space="PSUM") as psum_pool,
        ):
            # All inputs ride the sync ring (q1) in exact consumption order —
            # the two HWDGE rings don't round-robin fairly (q1 starves q10),
            # so FIFO position on q1 IS the data priority. Outputs go to the
            # scalar ring (q10), which naturally yields to input traffic.
            qt_tiles = [None] * K_TILES
            ptb = [[None] * K_TILES for _ in range(NBLK)]

            def load_front(k):
                fr_t = in_pool.tile([P, N_FRONT], in_dt, name=f"front_{k}")
                nc.sync.dma_start(out=fr_t, in_=qp[k * P : (k + 1) * P, 0:N_FRONT])
                qt_tiles[k] = fr_t[:, 0:ROWS]
                ptb[0][k] = fr_t[:, ROWS:N_FRONT]

            def load_b1(k):
                b1_t = in_pool.tile([P, NB], in_dt, name=f"b1_{k}")
                nc.sync.dma_start(
                    out=b1_t, in_=qp[k * P : (k + 1) * P, N_FRONT : N_FRONT + NB]
                )
                ptb[1][k] = b1_t

            def load_rest(k):
                re_t = in_pool.tile([P, N_REST], in_dt, name=f"rest_{k}")
                nc.sync.dma_start(
                    out=re_t, in_=qp[k * P : (k + 1) * P, N_FRONT + NB : ROWS + N_P]
                )
                for b in range(2, NBLK):
                    ptb[b][k] = re_t[:, (b - 2) * NB : (b - 1) * NB]

            load_front(0)
            # b row early (tiny); its on-device partition broadcast (saves a
            # 2 MiB HBM load) runs on GpSimd during the input stream.
            brow_t = in_pool.tile([1, N_P], f32, name="brow_t")
            nc.sync.dma_start(out=brow_t, in_=bb[0:1, :])
            bb_t = in_pool.tile([P, N_P], f32, name="bb_t")
            nc.gpsimd.partition_broadcast(bb_t, brow_t)
            for k in range(1, K_TILES):
                load_front(k)
            for k in range(K_TILES):
                load_b1(k)
            ab_t = in_pool.tile([P, M_TILES], f32, name="ab_t")
            nc.sync.dma_start(out=ab_t, in_=ab[:, :])
            for k in range(K_TILES):
                load_rest(k)

            n_out = 0

            def epilogue(psum_t, m, b):
                nonlocal n_out
                out_t = out_pool.tile([P, NB], f32, name="out_t")
                # out = (psum + xsq/D[i]) + ysq/D[j]
                nc.vector.scalar_tensor_tensor(
                    out=out_t,
                    in0=psum_t,
                    scalar=ab_t[:, m : m + 1],
                    in1=bb_t[:, b * NB : (b + 1) * NB],
                    op0=add,
                    op1=add,
                )
                out_eng = nc.scalar if n_out % 2 == 0 else nc.sync
                n_out += 1
                out_eng.dma_start(
                    out=out[m * P : (m + 1) * P, b * NB : (b + 1) * NB],
                    in_=out_t,
                )

            def mm(psum_t, m, b, k):
                nc.tensor.matmul(
                    psum_t,
                    mm_cast(qt_tiles[k][:, m * P : (m + 1) * P]),
                    mm_cast(ptb[b][k]),
                    start=(k == 0),
                    stop=(k == K_TILES - 1),
                )

            # Block 0: k-outer / m-inner over all 8 m-tiles (8 one-bank PSUM
            # groups) — each newly-landed chunk unlocks a full 8-matmul
            # sweep, so the PE goes dense while inputs are still streaming.
            psums = [
                psum_pool.tile([P, NB], f32, name="ps", tag="ps")
                for _ in range(M_TILES)
            ]
            for k in range(K_TILES):
                for m in range(M_TILES):
                    mm(psums[m], m, 0, k)
            for m in range(M_TILES):
                epilogue(psums[m], m, 0)

            # Blocks 1-7: everything is resident by then — m-outer / k-inner,
            # so each group's epilogue pipelines under the next group's
            # matmuls (no 8-deep epilogue pile-up blocking PSUM recycling),
            # and the kernel tail is a single small epilogue + 256 KiB store.
            for b in range(1, NBLK):
                for m in range(M_TILES):
                    psum_t = psum_pool.tile([P, NB], f32, name="ps", tag="ps")
                    for k in range(K_TILES):
                        mm(psum_t, m, b, k)
                    epilogue(psum_t, m, b)

    nc.compile()
    return nc


def _get_nc(compute_dt: str):
    if compute_dt not in _CACHE:
        _CACHE[compute_dt] = _build_nc(compute_dt)
    return _CACHE[compute_dt]


def _prep_inputs(z_queries: np.ndarray, class_prototypes: np.ndarray, compute_dt: str):
    import ml_dtypes

    np_in = ml_dtypes.bfloat16 if compute_dt == "bf16" else np.float32

    z = np.ascontiguousarray(z_queries, dtype=np.float32)
    p = np.ascontiguousarray(class_prototypes, dtype=np.float32)

    a = (z.astype(np.float64) ** 2).sum(axis=1) / D  # (N_Q,) ||x||^2 / D
    b = (p.astype(np.float64) ** 2).sum(axis=1) / D  # (N_P,) ||y||^2 / D

    pt = (p.T * np.float32(-2.0 / D)).astype(np_in)  # [D, N_P]
    bb = np.ascontiguousarray(b.astype(np.float32).reshape(1, N_P))  # [1, N_P]

    in_maps = []
    for c in range(N_CORES):
        sl = slice(c * ROWS, (c + 1) * ROWS)
        qt_c = z[sl].T.astype(np_in)  # [D, ROWS]
        qp_c = np.ascontiguousarray(np.concatenate([qt_c, pt], axis=1))  # [D, ROWS+N_P]
        ab_c = np.ascontiguousarray(
            a[sl].astype(np.float32).reshape(M_TILES, P).T
        )  # [P, M_TILES]
        in_maps.append({"qp": qp_c, "ab": ab_c, "bb": bb})
    return in_maps


def run(z_queries, class_prototypes, compute_dt=COMPUTE_DT, **spmd_kwargs):
    from concourse.bass_utils import run_bass_kernel_spmd

    nc = _get_nc(compute_dt)
    in_maps = _prep_inputs(z_queries, class_prototypes, compute_dt)
    res = run_bass_kernel_spmd(nc, in_maps, core_ids=list(range(N_CORES)), **spmd_kwargs)
    full = np.concatenate([r["out"] for r in res.results], axis=0)
    return full, res


def kernel(z_queries: np.ndarray, class_prototypes: np.ndarray) -> np.ndarray:
    full, _ = run(z_queries, class_prototypes)
    return full
